# revision 6
# baseline (speedup 1.0000x reference)
"""Trainium2 Bass kernel for single-head attention (B=8, N=2048, C=512).

Strategy: data-parallel over batch across the 8 NeuronCores — each core
computes one full batch sample.  All large matmuls run in fp8(e4m3) with
perf_mode=DoubleRow, which packs two K=128 contraction tiles into one
matmul at ~1.44x the bf16 rate.  Layout is chosen so NO on-device
transposes are needed:

  per core (b = core id):
    qT[d,n] = w_q @ x_b^T          (DoubleRow over c-pairs)
    kT[d,n] = w_k @ x_b^T
    v[m,d]  = x_b @ w_v^T
    ST[m,n] = kT^T-tiles @ qT      (scores transposed, unscaled)
    PT[m,n] = exp(SCALE*ST - ln4)  (ACT, PSUM -> SBUF fp8; 1/4 scale
                                    keeps exp below the e4m3 max of 240)
    avT[d,n] = sum_m v-tile^T @ PT (= (P@V)^T, unnormalized)
    s[n]    = ones^T @ (sum_m PT)  (PT summed on DVE, one matmul/chunk)
    yT[e,n] = w_p @ (avT/16)       (unnormalized projection, bf16 out)
  host: out[b] = yT^T * 16 / s[:,None] + v_f32 + b_proj
  (softmax normalization is linear in the row, so it commutes with the
   projection and is applied on the host; the 1/4 PT scale cancels in
   yT/s exactly)

DoubleRow operand layout: both matmul operands are 3D APs [128, 2, F]
where axis 1 selects the K-chunk pair member; SBUF "pair tiles" hold the
two 128-row K chunks side by side in the free dim.  The host pre-packs
x^T and the weights into that layout ([256, 2*cols] fp8 DRAM tensors).

Pipelining: the first QKV matmul only waits for ~0.5MB of fp8 DMA; AV
lags the score/exp pipeline by two m-tiles so it never stalls on ACT;
the projection of chunk ch-1 is emitted after the attention of chunk ch
as PE filler.
"""

import math

import ml_dtypes
import numpy as np

import concourse.bass as bass
import concourse.mybir as mybir
import concourse.tile as tile
from concourse import bacc
from concourse.bass_utils import run_bass_kernel_spmd

P = 128           # partitions
N = 2048          # tokens per batch sample
C = 512           # model dim
NT = N // P       # 16 token (m) tiles
MP = NT // 2      # 8 m-tile pairs
CT = C // P       # 4 dim tiles
CP = CT // 2      # 2 dim-tile pairs
FB = 512          # free-dim block (n-chunk)
NCH = N // FB     # 4 n-chunks
B = 8             # batch == number of cores
SCALE = C ** -0.5
PT_BIAS = -math.log(64.0)  # exp scaled by 1/64: e4m3 overflows at 240;
                           # max scaled score measured ~8.9 over all cores
AV_SCALE = 1.0 / 16.0      # (P@V) prescale into fp8; host multiplies back

F32 = mybir.dt.float32
F32R = mybir.dt.float32r
BF16 = mybir.dt.bfloat16
FP8 = mybir.dt.float8e4
NP_FP8 = ml_dtypes.float8_e4m3
EXP = mybir.ActivationFunctionType.Exp
DR = mybir.MatmulPerfMode.DoubleRow


def build():
    nc = bacc.Bacc("TRN2", target_bir_lowering=False, debug=False)

    # c-pair layout: row cp*128+p, col j*cols+f  <->  source row cp*256+j*128+p
    xdr = nc.dram_tensor("xdr", [2 * P, NCH * 2 * FB], FP8, kind="ExternalInput")
    wqd = nc.dram_tensor("wqd", [2 * P, 2 * C], FP8, kind="ExternalInput")
    wkd = nc.dram_tensor("wkd", [2 * P, 2 * C], FP8, kind="ExternalInput")
    wvd = nc.dram_tensor("wvd", [2 * P, 2 * C], FP8, kind="ExternalInput")
    wpd = nc.dram_tensor("wpd", [2 * P, 2 * C], FP8, kind="ExternalInput")
    yT = nc.dram_tensor("yT", [C, N], BF16, kind="ExternalOutput")    # scaled by 1/16
    sden = nc.dram_tensor("sden", [1, N], F32, kind="ExternalOutput")

    with tile.TileContext(nc) as tc:
        with (
            tc.tile_pool(name="sb", bufs=2) as sb,
            tc.tile_pool(name="ps", bufs=2, space="PSUM") as psp,
        ):
            ones_f32 = sb.tile([P, 1], F32, tag="ones_f32", bufs=1)
            nc.vector.memset(ones_f32, 1.0)
            ones_col = sb.tile([P, 1], F32R, tag="ones", bufs=1)
            nc.vector.tensor_copy(ones_col, ones_f32)
            bias_t = sb.tile([P, 1], F32, tag="bias", bufs=1)
            nc.vector.memset(bias_t, PT_BIAS)

            # warm the PE clock (HAM) with dummy matmuls while the first
            # DMAs stream in; results are discarded.  gpsimd memset: that
            # engine clears its preamble barriers earliest, so the warm
            # stream starts sooner; FD=256 keeps the total under the DMA
            # landing time so the real stream is never delayed.
            warm = sb.tile([P, 2 * P], BF16, tag="warm", bufs=1)
            nc.gpsimd.memset(warm, 0.0)
            pwarm = psp.tile([P, 2 * P], F32, tag="psc", bufs=4, name="pwarm")
            for i in range(10):
                nc.tensor.matmul(pwarm, warm[:, 0:P], warm,
                                 start=True, stop=True)

            # ---- input loads, most-urgent first, split across the two
            # HWDGE queues (SP and Activation) so they land in parallel ----
            xts = {}
            wq = []
            for cp in range(CP):
                t = sb.tile([P, 2, C], FP8, tag="w", bufs=6, name=f"wwq{cp}")
                nc.sync.dma_start(t, wqd[cp * P:(cp + 1) * P, :])
                wq.append(t)
                t2 = sb.tile([P, 2, FB], FP8, tag="xt", bufs=8,
                             name=f"xt{cp}_0")
                nc.scalar.dma_start(t2, xdr[cp * P:(cp + 1) * P, 0:2 * FB])
                xts[(cp, 0)] = t2

            def load_pair_w(handle, tag, bufs, eng):
                ws = []
                for cp in range(CP):
                    t = sb.tile([P, 2, C], FP8, tag=tag, bufs=bufs,
                                name=f"w{handle.name}{cp}")
                    eng.dma_start(t, handle[cp * P:(cp + 1) * P, :])
                    ws.append(t)
                return ws

            wk = load_pair_w(wkd, "w", 6, nc.sync)
            wv = load_pair_w(wvd, "w", 6, nc.scalar)
            for ch in range(1, NCH):
                for cp in range(CP):
                    t = sb.tile([P, 2, FB], FP8, tag="xt", bufs=8,
                                name=f"xt{cp}_{ch}")
                    eng = nc.sync if (ch * CP + cp) % 2 == 0 else nc.scalar
                    eng.dma_start(
                        t, xdr[cp * P:(cp + 1) * P, ch * 2 * FB:(ch + 1) * 2 * FB])
                    xts[(cp, ch)] = t
            wp = load_pair_w(wpd, "wp", 2, nc.scalar)

            # ---- QKV projections, chunk-outer ----
            qts, kts, vs = {}, {}, {}
            for ch in range(NCH):
                for wt, store, nm in ((wq, qts, "q"), (wk, kts, "k")):
                    for dp in range(CP):
                        store[(dp, ch)] = sb.tile(
                            [P, 2, FB], FP8, tag="qk", bufs=16,
                            name=f"{nm}{dp}_{ch}")
                    for dt in range(CT):
                        ps = psp.tile([P, FB], F32, tag="psc", bufs=4,
                                      name=f"p{nm}{dt}_{ch}")
                        for cp in range(CP):
                            nc.tensor.matmul(
                                ps,
                                wt[cp][:, :, dt * P:(dt + 1) * P],
                                xts[(cp, ch)],
                                start=(cp == 0), stop=(cp == CP - 1),
                                perf_mode=DR,
                            )
                        dest = store[(dt // 2, ch)][:, dt % 2, :]
                        if nm == "q":
                            nc.vector.tensor_copy(dest, ps)
                        else:
                            nc.scalar.copy(dest, ps)
                for mi in range(ch * 4, ch * 4 + 4):
                    ps = psp.tile([P, C], F32, tag="pav", bufs=4,
                                  name=f"pv{mi}")
                    for cp in range(CP):
                        nc.tensor.matmul(
                            ps,
                            xts[(cp, ch)][:, :, (mi % 4) * P:(mi % 4 + 1) * P],
                            wv[cp],
                            start=(cp == 0), stop=(cp == CP - 1),
                            perf_mode=DR,
                        )
                    if mi % 2 == 0:
                        vs[mi // 2] = sb.tile([P, 2, C], FP8, tag="v", bufs=8,
                                              name=f"v{mi // 2}")
                    if mi % 2 == 0:
                        nc.vector.tensor_copy(vs[mi // 2][:, mi % 2, :], ps)
                    else:
                        nc.scalar.copy(vs[mi // 2][:, mi % 2, :], ps)

            # ---- attention per n-chunk; proj(ch-1) emitted after
            # attention(ch) so it fills PE bubbles ----
            saved = {}

            def emit_proj(ch, avts):
                for et in range(CT):
                    py = psp.tile([P, FB], F32, tag="psc", bufs=4,
                                  name=f"py{et}_{ch}")
                    for dp in range(CP):
                        nc.tensor.matmul(
                            py,
                            wp[dp][:, :, et * P:(et + 1) * P],
                            avts[dp],
                            start=(dp == 0), stop=(dp == CP - 1),
                            perf_mode=DR,
                        )
                    yt = sb.tile([P, FB], BF16, tag="yo", bufs=3,
                                 name=f"yt{et}_{ch}")
                    if ch == NCH - 1 and et % 2 == 1:
                        nc.scalar.copy(yt, py)
                    else:
                        nc.vector.tensor_copy(yt, py)
                    nc.sync.dma_start(
                        yT[et * P:(et + 1) * P, ch * FB:(ch + 1) * FB], yt)

            for ch in range(NCH):
                pavs = [
                    psp.tile([P, FB], F32, tag="pav", bufs=4,
                             name=f"pav{ch}_{dt}")
                    for dt in range(CT)
                ]
                acc_s = sb.tile([P, FB], F32R, tag="accs", bufs=2,
                                name=f"accs{ch}")
                pts = {}

                def emit_av(mp):
                    pt = pts.pop(mp)
                    for dt in range(CT):
                        nc.tensor.matmul(
                            pavs[dt],
                            vs[mp][:, :, dt * P:(dt + 1) * P],
                            pt,
                            start=(mp == 0), stop=(mp == MP - 1),
                            perf_mode=DR,
                        )

                for mi in range(NT):
                    psc = psp.tile([P, FB], F32, tag="psc", bufs=4,
                                   name=f"psc{ch}_{mi}")
                    for dp in range(CP):
                        nc.tensor.matmul(
                            psc,
                            kts[(dp, mi // 4)][:, :, (mi % 4) * P:(mi % 4 + 1) * P],
                            qts[(dp, ch)],
                            start=(dp == 0), stop=(dp == CP - 1),
                            perf_mode=DR,
                        )
                    if mi % 2 == 0:
                        pts[mi // 2] = sb.tile([P, 2, FB], FP8, tag="pt",
                                               bufs=16, name=f"pt{ch}_{mi // 2}")
                    dest = pts[mi // 2][:, mi % 2, :]
                    nc.scalar.activation(dest, psc, EXP,
                                         bias=bias_t, scale=SCALE)
                    if mi == 0:
                        nc.vector.tensor_copy(acc_s, dest)
                    else:
                        nc.vector.tensor_add(acc_s, acc_s, dest)
                    # AV lags the exp pipeline by one pair (two m-tiles) so
                    # it never stalls on ACT latency
                    if mi >= 3 and mi % 2 == 1:
                        emit_av((mi - 3) // 2)
                emit_av(MP - 1)

                avts = []
                for dp in range(CP):
                    avts.append(sb.tile([P, 2, FB], FP8, tag="avt", bufs=4,
                                        name=f"avt{ch}_{dp}"))
                for dt in range(CT):
                    # alternate engines so both halves of each pair land in
                    # parallel and the trailing proj never stalls on them
                    dest = avts[dt // 2][:, dt % 2, :]
                    if dt % 2 == 0:
                        nc.scalar.mul(dest, pavs[dt], AV_SCALE)
                    else:
                        nc.vector.tensor_scalar_mul(dest, pavs[dt], AV_SCALE)
                saved[ch] = avts

                ps_s = psp.tile([1, FB], F32, tag="psc", bufs=4,
                                name=f"ps_s{ch}")
                nc.tensor.matmul(ps_s, ones_col, acc_s, start=True, stop=True)
                s_sb = sb.tile([1, FB], F32, tag="s", bufs=4, name=f"s{ch}")
                nc.vector.tensor_copy(s_sb, ps_s)
                nc.sync.dma_start(sden[:, ch * FB:(ch + 1) * FB], s_sb)

                if ch > 0:
                    emit_proj(ch - 1, saved.pop(ch - 1))
            emit_proj(NCH - 1, saved.pop(NCH - 1))

    nc.compile()
    return nc


def _pack_pairs(a):
    """[512, F] -> [256, 2F] c-pair layout (rows cp*256+j*128+p)."""
    f = a.shape[1]
    return np.ascontiguousarray(
        a.reshape(2, 2, P, f).transpose(0, 2, 1, 3).reshape(2 * P, 2 * f))


def _pack_x(xT):
    """[512, 2048] -> [256, 4096]: per chunk ch, cols ch*1024+j*512+f."""
    return np.ascontiguousarray(
        xT.reshape(2, 2, P, NCH, FB).transpose(0, 2, 3, 1, 4)
        .reshape(2 * P, NCH * 2 * FB))


def _prep_in_maps(x, w_qkv):
    wq = _pack_pairs(w_qkv[0:C].T.astype(np.float32)).astype(NP_FP8)
    wk = _pack_pairs(w_qkv[C:2 * C].T.astype(np.float32)).astype(NP_FP8)
    wv = _pack_pairs(w_qkv[2 * C:3 * C].T.astype(np.float32)).astype(NP_FP8)
    in_maps = []
    for b in range(B):
        in_maps.append({
            "xdr": _pack_x(x[b].T.astype(np.float32)).astype(NP_FP8),
            "wqd": wq, "wkd": wk, "wvd": wv,
        })
    return in_maps


_NC = None


def _get_nc():
    global _NC
    if _NC is None:
        _NC = build()
    return _NC


def kernel(x, w_qkv, w_proj, b_proj):
    x = np.asarray(x, dtype=np.float32)
    w_qkv = np.asarray(w_qkv, dtype=np.float32)
    w_proj = np.asarray(w_proj, dtype=np.float32)
    b_proj = np.asarray(b_proj, dtype=np.float32)

    in_maps = _prep_in_maps(x, w_qkv)
    wpp = _pack_pairs(w_proj.T).astype(NP_FP8)
    for m in in_maps:
        m["wpd"] = wpp

    nc = _get_nc()
    res = None
    for attempt in range(3):
        try:
            res = run_bass_kernel_spmd(nc, in_maps, core_ids=list(range(B)))
            break
        except Exception:
            if attempt == 2:
                raise
            import time
            time.sleep(5)

    wv_f32 = w_qkv[2 * C:3 * C]
    out = np.empty((B, N, C), np.float32)
    for b in range(B):
        r = res.results[b]
        s = r["sden"].reshape(N, 1)
        yt = np.asarray(r["yT"]).astype(np.float32)
        out[b] = yt.T * (1.0 / AV_SCALE) / s + (x[b] @ wv_f32.T) + b_proj[None, :]
    return out


# revision 7
# speedup vs baseline: 1.2772x; 1.2772x over previous
"""Trainium2 Bass kernel for single-head attention (B=8, N=2048, C=512).

Strategy: data-parallel over batch across the 8 NeuronCores — each core
computes one full batch sample.  All large matmuls run in fp8(e4m3) with
perf_mode=DoubleRow, which packs two K=128 contraction tiles into one
matmul at ~1.44x the bf16 rate.  Layout is chosen so NO on-device
transposes are needed:

  per core (b = core id):
    qT[d,n] = w_q @ x_b^T          (DoubleRow over c-pairs)
    kT[d,n] = w_k @ x_b^T
    v[m,d]  = x_b @ w_v^T
    ST[m,n] = kT^T-tiles @ qT      (scores transposed, unscaled)
    PT[m,n] = exp(SCALE*ST - ln4)  (ACT, PSUM -> SBUF fp8; 1/4 scale
                                    keeps exp below the e4m3 max of 240)
    avT[d,n] = sum_m v-tile^T @ PT (= (P@V)^T, unnormalized)
    s[n]    = ones^T @ (sum_m PT)  (PT summed on DVE, one matmul/chunk)
    yT[e,n] = w_p @ (avT/16)       (unnormalized projection, bf16 out)
  host: out[b] = yT^T * 16 / s[:,None] + v_f32 + b_proj
  (softmax normalization is linear in the row, so it commutes with the
   projection and is applied on the host; the 1/4 PT scale cancels in
   yT/s exactly)

DoubleRow operand layout: both matmul operands are 3D APs [128, 2, F]
where axis 1 selects the K-chunk pair member; SBUF "pair tiles" hold the
two 128-row K chunks side by side in the free dim.  The host pre-packs
x^T and the weights into that layout ([256, 2*cols] fp8 DRAM tensors).

Pipelining: the first QKV matmul only waits for ~0.5MB of fp8 DMA; AV
lags the score/exp pipeline by two m-tiles so it never stalls on ACT;
the projection of chunk ch-1 is emitted after the attention of chunk ch
as PE filler.
"""

import math

import ml_dtypes
import numpy as np

import concourse.bass as bass
import concourse.mybir as mybir
import concourse.tile as tile
from concourse import bacc
from concourse.bass_utils import run_bass_kernel_spmd

P = 128           # partitions
N = 2048          # tokens per batch sample
C = 512           # model dim
NT = N // P       # 16 token (m) tiles
MP = NT // 2      # 8 m-tile pairs
CT = C // P       # 4 dim tiles
CP = CT // 2      # 2 dim-tile pairs
FB = 512          # free-dim block (n-chunk)
NCH = N // FB     # 4 n-chunks
B = 8             # batch == number of cores
SCALE = C ** -0.5
PT_BIAS = -math.log(64.0)  # exp scaled by 1/64: e4m3 overflows at 240;
                           # max scaled score measured ~8.9 over all cores
AV_SCALE = 1.0 / 16.0      # (P@V) prescale into fp8; host multiplies back

F32 = mybir.dt.float32
F32R = mybir.dt.float32r
BF16 = mybir.dt.bfloat16
FP8 = mybir.dt.float8e4
NP_FP8 = ml_dtypes.float8_e4m3
EXP = mybir.ActivationFunctionType.Exp
DR = mybir.MatmulPerfMode.DoubleRow


def build():
    nc = bacc.Bacc("TRN2", target_bir_lowering=False, debug=False)

    # c-pair layout: row cp*128+p, col j*cols+f  <->  source row cp*256+j*128+p
    xdr = nc.dram_tensor("xdr", [2 * P, NCH * 2 * FB], FP8, kind="ExternalInput")
    wqd = nc.dram_tensor("wqd", [2 * P, 2 * C], FP8, kind="ExternalInput")
    wkd = nc.dram_tensor("wkd", [2 * P, 2 * C], FP8, kind="ExternalInput")
    wvd = nc.dram_tensor("wvd", [2 * P, 2 * C], FP8, kind="ExternalInput")
    wpd = nc.dram_tensor("wpd", [2 * P, 2 * C], FP8, kind="ExternalInput")
    yT = nc.dram_tensor("yT", [C, N], BF16, kind="ExternalOutput")    # scaled by 1/16
    sden = nc.dram_tensor("sden", [1, N], F32, kind="ExternalOutput")

    with tile.TileContext(nc) as tc:
        with (
            tc.tile_pool(name="sb", bufs=2) as sb,
            tc.tile_pool(name="ps", bufs=2, space="PSUM") as psp,
        ):
            ones_f32 = sb.tile([P, 1], F32, tag="ones_f32", bufs=1)
            nc.vector.memset(ones_f32, 1.0)
            ones_col = sb.tile([P, 1], F32R, tag="ones", bufs=1)
            nc.vector.tensor_copy(ones_col, ones_f32)
            bias_t = sb.tile([P, 1], F32, tag="bias", bufs=1)
            nc.vector.memset(bias_t, PT_BIAS)

            # warm the PE clock (HAM) with dummy matmuls while the first
            # DMAs stream in; results are discarded.  gpsimd memset: that
            # engine clears its preamble barriers earliest, so the warm
            # stream starts sooner; FD=256 keeps the total under the DMA
            # landing time so the real stream is never delayed.
            warm = sb.tile([P, 2 * P], BF16, tag="warm", bufs=1)
            nc.gpsimd.memset(warm, 0.0)
            pwarm = psp.tile([P, 2 * P], F32, tag="psc", bufs=4, name="pwarm")
            for i in range(10):
                nc.tensor.matmul(pwarm, warm[:, 0:P], warm,
                                 start=True, stop=True)

            # ---- input loads, most-urgent first, split across the two
            # HWDGE queues (SP and Activation) so they land in parallel ----
            xts = {}
            wq = []
            for cp in range(CP):
                t = sb.tile([P, 2, C], FP8, tag="w", bufs=6, name=f"wwq{cp}")
                nc.sync.dma_start(t, wqd[cp * P:(cp + 1) * P, :])
                wq.append(t)
                t2 = sb.tile([P, 2, FB], FP8, tag="xt", bufs=8,
                             name=f"xt{cp}_0")
                nc.scalar.dma_start(t2, xdr[cp * P:(cp + 1) * P, 0:2 * FB])
                xts[(cp, 0)] = t2

            def load_pair_w(handle, tag, bufs, eng):
                ws = []
                for cp in range(CP):
                    t = sb.tile([P, 2, C], FP8, tag=tag, bufs=bufs,
                                name=f"w{handle.name}{cp}")
                    eng.dma_start(t, handle[cp * P:(cp + 1) * P, :])
                    ws.append(t)
                return ws

            wk = load_pair_w(wkd, "w", 6, nc.sync)
            wv = load_pair_w(wvd, "w", 6, nc.scalar)
            for ch in range(1, NCH):
                for cp in range(CP):
                    t = sb.tile([P, 2, FB], FP8, tag="xt", bufs=8,
                                name=f"xt{cp}_{ch}")
                    eng = nc.sync if (ch * CP + cp) % 2 == 0 else nc.scalar
                    eng.dma_start(
                        t, xdr[cp * P:(cp + 1) * P, ch * 2 * FB:(ch + 1) * 2 * FB])
                    xts[(cp, ch)] = t
            wp = load_pair_w(wpd, "wp", 2, nc.scalar)

            # ---- QKV projections, chunk-outer ----
            qts, kts, vs = {}, {}, {}
            for ch in range(NCH):
                for wt, store, nm in ((wq, qts, "q"), (wk, kts, "k")):
                    for dp in range(CP):
                        store[(dp, ch)] = sb.tile(
                            [P, 2, FB], FP8, tag="qk", bufs=16,
                            name=f"{nm}{dp}_{ch}")
                    for dt in range(CT):
                        ps = psp.tile([P, FB], F32, tag="psc", bufs=4,
                                      name=f"p{nm}{dt}_{ch}")
                        for cp in range(CP):
                            nc.tensor.matmul(
                                ps,
                                wt[cp][:, :, dt * P:(dt + 1) * P],
                                xts[(cp, ch)],
                                start=(cp == 0), stop=(cp == CP - 1),
                                perf_mode=DR,
                            )
                        dest = store[(dt // 2, ch)][:, dt % 2, :]
                        if nm == "q":
                            nc.vector.tensor_copy(dest, ps)
                        else:
                            nc.scalar.copy(dest, ps)
                for mi in range(ch * 4, ch * 4 + 4):
                    ps = psp.tile([P, C], F32, tag="pav", bufs=4,
                                  name=f"pv{mi}")
                    for cp in range(CP):
                        nc.tensor.matmul(
                            ps,
                            xts[(cp, ch)][:, :, (mi % 4) * P:(mi % 4 + 1) * P],
                            wv[cp],
                            start=(cp == 0), stop=(cp == CP - 1),
                            perf_mode=DR,
                        )
                    if mi % 2 == 0:
                        vs[mi // 2] = sb.tile([P, 2, C], FP8, tag="v", bufs=8,
                                              name=f"v{mi // 2}")
                    if mi % 2 == 0:
                        nc.vector.tensor_copy(vs[mi // 2][:, mi % 2, :], ps)
                    else:
                        nc.scalar.copy(vs[mi // 2][:, mi % 2, :], ps)

            # ---- attention per n-chunk; proj(ch-1) emitted after
            # attention(ch) so it fills PE bubbles ----
            saved = {}

            def emit_proj(ch, avts):
                for et in range(CT):
                    py = psp.tile([P, FB], F32, tag="psc", bufs=4,
                                  name=f"py{et}_{ch}")
                    for dp in range(CP):
                        nc.tensor.matmul(
                            py,
                            wp[dp][:, :, et * P:(et + 1) * P],
                            avts[dp],
                            start=(dp == 0), stop=(dp == CP - 1),
                            perf_mode=DR,
                        )
                    yt = sb.tile([P, FB], BF16, tag="yo", bufs=3,
                                 name=f"yt{et}_{ch}")
                    if ch == NCH - 1 and et % 2 == 1:
                        nc.scalar.copy(yt, py)
                    else:
                        nc.vector.tensor_copy(yt, py)
                    nc.sync.dma_start(
                        yT[et * P:(et + 1) * P, ch * FB:(ch + 1) * FB], yt)

            for ch in range(NCH):
                pavs = [
                    psp.tile([P, FB], F32, tag="pav", bufs=4,
                             name=f"pav{ch}_{dt}")
                    for dt in range(CT)
                ]
                acc_s = sb.tile([P, FB], F32R, tag="accs", bufs=2,
                                name=f"accs{ch}")
                pts = {}

                def emit_av(mp):
                    pt = pts.pop(mp)
                    for dt in range(CT):
                        nc.tensor.matmul(
                            pavs[dt],
                            vs[mp][:, :, dt * P:(dt + 1) * P],
                            pt,
                            start=(mp == 0), stop=(mp == MP - 1),
                            perf_mode=DR,
                        )

                for mi in range(NT):
                    psc = psp.tile([P, FB], F32, tag="psc", bufs=4,
                                   name=f"psc{ch}_{mi}")
                    for dp in range(CP):
                        nc.tensor.matmul(
                            psc,
                            kts[(dp, mi // 4)][:, :, (mi % 4) * P:(mi % 4 + 1) * P],
                            qts[(dp, ch)],
                            start=(dp == 0), stop=(dp == CP - 1),
                            perf_mode=DR,
                        )
                    if mi % 2 == 0:
                        pts[mi // 2] = sb.tile([P, 2, FB], FP8, tag="pt",
                                               bufs=16, name=f"pt{ch}_{mi // 2}")
                    dest = pts[mi // 2][:, mi % 2, :]
                    nc.scalar.activation(dest, psc, EXP,
                                         bias=bias_t, scale=SCALE)
                    if mi == 0:
                        nc.vector.tensor_copy(acc_s, dest)
                    else:
                        nc.vector.tensor_add(acc_s, acc_s, dest)
                    # AV lags the exp pipeline by one pair (two m-tiles) so
                    # it never stalls on ACT latency
                    if mi >= 3 and mi % 2 == 1:
                        emit_av((mi - 3) // 2)
                emit_av(MP - 1)

                avts = []
                for dp in range(CP):
                    avts.append(sb.tile([P, 2, FB], FP8, tag="avt", bufs=4,
                                        name=f"avt{ch}_{dp}"))
                for dt in range(CT):
                    nc.scalar.mul(avts[dt // 2][:, dt % 2, :], pavs[dt],
                                  AV_SCALE)
                saved[ch] = avts

                if ch > 0:
                    emit_proj(ch - 1, saved.pop(ch - 1))

                # the sden matmul waits on the full 16-add DVE chain, so it
                # is emitted AFTER the proj matmuls — keeping that chain off
                # the PE critical path at the chunk boundary
                ps_s = psp.tile([1, FB], F32, tag="psc", bufs=4,
                                name=f"ps_s{ch}")
                nc.tensor.matmul(ps_s, ones_col, acc_s, start=True, stop=True)
                s_sb = sb.tile([1, FB], F32, tag="s", bufs=4, name=f"s{ch}")
                nc.vector.tensor_copy(s_sb, ps_s)
                nc.sync.dma_start(sden[:, ch * FB:(ch + 1) * FB], s_sb)
            emit_proj(NCH - 1, saved.pop(NCH - 1))

    nc.compile()
    return nc


def _pack_pairs(a):
    """[512, F] -> [256, 2F] c-pair layout (rows cp*256+j*128+p)."""
    f = a.shape[1]
    return np.ascontiguousarray(
        a.reshape(2, 2, P, f).transpose(0, 2, 1, 3).reshape(2 * P, 2 * f))


def _pack_x(xT):
    """[512, 2048] -> [256, 4096]: per chunk ch, cols ch*1024+j*512+f."""
    return np.ascontiguousarray(
        xT.reshape(2, 2, P, NCH, FB).transpose(0, 2, 3, 1, 4)
        .reshape(2 * P, NCH * 2 * FB))


def _prep_in_maps(x, w_qkv):
    wq = _pack_pairs(w_qkv[0:C].T.astype(np.float32)).astype(NP_FP8)
    wk = _pack_pairs(w_qkv[C:2 * C].T.astype(np.float32)).astype(NP_FP8)
    wv = _pack_pairs(w_qkv[2 * C:3 * C].T.astype(np.float32)).astype(NP_FP8)
    in_maps = []
    for b in range(B):
        in_maps.append({
            "xdr": _pack_x(x[b].T.astype(np.float32)).astype(NP_FP8),
            "wqd": wq, "wkd": wk, "wvd": wv,
        })
    return in_maps


_NC = None


def _get_nc():
    global _NC
    if _NC is None:
        _NC = build()
    return _NC


def kernel(x, w_qkv, w_proj, b_proj):
    x = np.asarray(x, dtype=np.float32)
    w_qkv = np.asarray(w_qkv, dtype=np.float32)
    w_proj = np.asarray(w_proj, dtype=np.float32)
    b_proj = np.asarray(b_proj, dtype=np.float32)

    in_maps = _prep_in_maps(x, w_qkv)
    wpp = _pack_pairs(w_proj.T).astype(NP_FP8)
    for m in in_maps:
        m["wpd"] = wpp

    nc = _get_nc()
    res = None
    for attempt in range(3):
        try:
            res = run_bass_kernel_spmd(nc, in_maps, core_ids=list(range(B)))
            break
        except Exception:
            if attempt == 2:
                raise
            import time
            time.sleep(5)

    wv_f32 = w_qkv[2 * C:3 * C]
    out = np.empty((B, N, C), np.float32)
    for b in range(B):
        r = res.results[b]
        s = r["sden"].reshape(N, 1)
        yt = np.asarray(r["yT"]).astype(np.float32)
        out[b] = yt.T * (1.0 / AV_SCALE) / s + (x[b] @ wv_f32.T) + b_proj[None, :]
    return out


# revision 10
# speedup vs baseline: 1.3037x; 1.0208x over previous
"""Trainium2 Bass kernel for single-head attention (B=8, N=2048, C=512).

Strategy: data-parallel over batch across the 8 NeuronCores — each core
computes one full batch sample.  All large matmuls run in fp8(e4m3) with
perf_mode=DoubleRow, which packs two K=128 contraction tiles into one
matmul at ~1.44x the bf16 rate.  Layout is chosen so NO on-device
transposes are needed:

  per core (b = core id):
    qT[d,n] = w_q @ x_b^T          (DoubleRow over c-pairs)
    kT[d,n] = w_k @ x_b^T
    v[m,d]  = x_b @ w_v^T
    ST[m,n] = kT^T-tiles @ qT      (scores transposed, unscaled)
    PT[m,n] = exp(SCALE*ST - ln4)  (ACT, PSUM -> SBUF fp8; 1/4 scale
                                    keeps exp below the e4m3 max of 240)
    avT[d,n] = sum_m v-tile^T @ PT (= (P@V)^T, unnormalized)
    s[n]    = ones^T @ (sum_m PT)  (PT summed on DVE, one matmul/chunk)
    yT[e,n] = w_p @ (avT/16)       (unnormalized projection, bf16 out)
  host: out[b] = yT^T * 16 / s[:,None] + v_f32 + b_proj
  (softmax normalization is linear in the row, so it commutes with the
   projection and is applied on the host; the 1/4 PT scale cancels in
   yT/s exactly)

DoubleRow operand layout: both matmul operands are 3D APs [128, 2, F]
where axis 1 selects the K-chunk pair member; SBUF "pair tiles" hold the
two 128-row K chunks side by side in the free dim.  The host pre-packs
x^T and the weights into that layout ([256, 2*cols] fp8 DRAM tensors).

Pipelining: the first QKV matmul only waits for ~0.5MB of fp8 DMA; AV
lags the score/exp pipeline by two m-tiles so it never stalls on ACT;
the projection of chunk ch-1 is emitted after the attention of chunk ch
as PE filler.
"""

import math

import ml_dtypes
import numpy as np

import concourse.bass as bass
import concourse.mybir as mybir
import concourse.tile as tile
from concourse import bacc
from concourse.bass_utils import run_bass_kernel_spmd

P = 128           # partitions
N = 2048          # tokens per batch sample
C = 512           # model dim
NT = N // P       # 16 token (m) tiles
MP = NT // 2      # 8 m-tile pairs
CT = C // P       # 4 dim tiles
CP = CT // 2      # 2 dim-tile pairs
FB = 512          # free-dim block (n-chunk)
NCH = N // FB     # 4 n-chunks
B = 8             # batch == number of cores
SCALE = C ** -0.5
PT_BIAS = -math.log(64.0)  # exp scaled by 1/64: e4m3 overflows at 240;
                           # max scaled score measured ~8.9 over all cores
AV_SCALE = 1.0 / 16.0      # (P@V) prescale into fp8; host multiplies back

F32 = mybir.dt.float32
F32R = mybir.dt.float32r
BF16 = mybir.dt.bfloat16
FP8 = mybir.dt.float8e4
NP_FP8 = ml_dtypes.float8_e4m3
EXP = mybir.ActivationFunctionType.Exp
DR = mybir.MatmulPerfMode.DoubleRow


def build():
    nc = bacc.Bacc("TRN2", target_bir_lowering=False, debug=False)

    # c-pair layout: row cp*128+p, col j*cols+f  <->  source row cp*256+j*128+p
    xdr = nc.dram_tensor("xdr", [2 * P, NCH * 2 * FB], FP8, kind="ExternalInput")
    wqd = nc.dram_tensor("wqd", [2 * P, 2 * C], FP8, kind="ExternalInput")
    wkd = nc.dram_tensor("wkd", [2 * P, 2 * C], FP8, kind="ExternalInput")
    wvd = nc.dram_tensor("wvd", [2 * P, 2 * C], FP8, kind="ExternalInput")
    wpd = nc.dram_tensor("wpd", [2 * P, 2 * C], FP8, kind="ExternalInput")
    yT = nc.dram_tensor("yT", [C, N], BF16, kind="ExternalOutput")    # scaled by 1/16
    sden = nc.dram_tensor("sden", [1, N], F32, kind="ExternalOutput")

    with tile.TileContext(nc) as tc:
        with (
            tc.tile_pool(name="sb", bufs=2) as sb,
            tc.tile_pool(name="ps", bufs=2, space="PSUM") as psp,
        ):
            ones_f32 = sb.tile([P, 1], F32, tag="ones_f32", bufs=1)
            nc.vector.memset(ones_f32, 1.0)
            ones_col = sb.tile([P, 1], F32R, tag="ones", bufs=1)
            nc.vector.tensor_copy(ones_col, ones_f32)
            bias_t = sb.tile([P, 1], F32, tag="bias", bufs=1)
            nc.vector.memset(bias_t, PT_BIAS)

            # warm the PE clock (HAM) with dummy matmuls while the first
            # DMAs stream in; results are discarded.  gpsimd memset: that
            # engine clears its preamble barriers earliest, so the warm
            # stream starts sooner; FD=256 keeps the total under the DMA
            # landing time so the real stream is never delayed.
            warm = sb.tile([P, 2 * P], BF16, tag="warm", bufs=1)
            nc.gpsimd.memset(warm, 0.0)
            pwarm = psp.tile([P, 2 * P], F32, tag="psc", bufs=4, name="pwarm")
            for i in range(10):
                nc.tensor.matmul(pwarm, warm[:, 0:P], warm,
                                 start=True, stop=True)

            # ---- input loads, most-urgent first.  The two x chunk-0 tiles
            # go on the Activation HWDGE queue (empty at this point) so they
            # land in parallel with the weights on the SP queue; everything
            # else stays on SP to keep the ACT queue free for exp/copies ----
            xts = {}
            for cp in range(CP):
                t2 = sb.tile([P, 2, FB], FP8, tag="xt", bufs=8,
                             name=f"xt{cp}_0")
                nc.scalar.dma_start(t2, xdr[cp * P:(cp + 1) * P, 0:2 * FB])
                xts[(cp, 0)] = t2

            def load_pair_w(handle, tag, bufs):
                ws = []
                for cp in range(CP):
                    t = sb.tile([P, 2, C], FP8, tag=tag, bufs=bufs,
                                name=f"w{handle.name}{cp}")
                    nc.sync.dma_start(t, handle[cp * P:(cp + 1) * P, :])
                    ws.append(t)
                return ws

            wq = load_pair_w(wqd, "w", 6)
            wk = load_pair_w(wkd, "w", 6)
            wv = load_pair_w(wvd, "w", 6)
            for ch in range(1, NCH):
                for cp in range(CP):
                    t = sb.tile([P, 2, FB], FP8, tag="xt", bufs=8,
                                name=f"xt{cp}_{ch}")
                    nc.sync.dma_start(
                        t, xdr[cp * P:(cp + 1) * P, ch * 2 * FB:(ch + 1) * 2 * FB])
                    xts[(cp, ch)] = t
            wp = load_pair_w(wpd, "wp", 2)

            # ---- QKV projections, chunk-outer ----
            qts, kts, vs = {}, {}, {}
            for ch in range(NCH):
                for wt, store, nm in ((wq, qts, "q"), (wk, kts, "k")):
                    for dp in range(CP):
                        store[(dp, ch)] = sb.tile(
                            [P, 2, FB], FP8, tag="qk", bufs=16,
                            name=f"{nm}{dp}_{ch}")
                    for dt in range(CT):
                        ps = psp.tile([P, FB], F32, tag="psc", bufs=4,
                                      name=f"p{nm}{dt}_{ch}")
                        for cp in range(CP):
                            nc.tensor.matmul(
                                ps,
                                wt[cp][:, :, dt * P:(dt + 1) * P],
                                xts[(cp, ch)],
                                start=(cp == 0), stop=(cp == CP - 1),
                                perf_mode=DR,
                            )
                        dest = store[(dt // 2, ch)][:, dt % 2, :]
                        if nm == "q":
                            nc.vector.tensor_copy(dest, ps)
                        else:
                            nc.scalar.copy(dest, ps)
                for mi in range(ch * 4, ch * 4 + 4):
                    ps = psp.tile([P, C], F32, tag="pav", bufs=4,
                                  name=f"pv{mi}")
                    for cp in range(CP):
                        nc.tensor.matmul(
                            ps,
                            xts[(cp, ch)][:, :, (mi % 4) * P:(mi % 4 + 1) * P],
                            wv[cp],
                            start=(cp == 0), stop=(cp == CP - 1),
                            perf_mode=DR,
                        )
                    if mi % 2 == 0:
                        vs[mi // 2] = sb.tile([P, 2, C], FP8, tag="v", bufs=8,
                                              name=f"v{mi // 2}")
                    if mi % 2 == 0:
                        nc.vector.tensor_copy(vs[mi // 2][:, mi % 2, :], ps)
                    else:
                        nc.scalar.copy(vs[mi // 2][:, mi % 2, :], ps)

            # ---- attention per n-chunk.  proj(ch-1) and sden(ch-1) are
            # interleaved INTO chunk ch's score/AV stream (one proj column
            # tile every other m-tile) so neither ever sits on the PE
            # critical path at a chunk boundary ----
            saved = {}

            def emit_proj_et(ch, avts, et, last=False):
                py = psp.tile([P, FB], F32, tag="psc", bufs=4,
                              name=f"py{et}_{ch}")
                for dp in range(CP):
                    nc.tensor.matmul(
                        py,
                        wp[dp][:, :, et * P:(et + 1) * P],
                        avts[dp],
                        start=(dp == 0), stop=(dp == CP - 1),
                        perf_mode=DR,
                    )
                yt = sb.tile([P, FB], BF16, tag="yo", bufs=3,
                             name=f"yt{et}_{ch}")
                if et % 2 == 1 or (last and et % 2 == 0):
                    nc.scalar.copy(yt, py)
                else:
                    nc.vector.tensor_copy(yt, py)
                nc.sync.dma_start(
                    yT[et * P:(et + 1) * P, ch * FB:(ch + 1) * FB], yt)

            def emit_sden(ch, acc):
                ps_s = psp.tile([1, FB], F32, tag="psc", bufs=4,
                                name=f"ps_s{ch}")
                nc.tensor.matmul(ps_s, ones_col, acc, start=True, stop=True)
                s_sb = sb.tile([1, FB], F32, tag="s", bufs=4, name=f"s{ch}")
                nc.vector.tensor_copy(s_sb, ps_s)
                nc.sync.dma_start(sden[:, ch * FB:(ch + 1) * FB], s_sb)

            prev_acc = None
            for ch in range(NCH):
                pavs = [
                    psp.tile([P, FB], F32, tag="pav", bufs=4,
                             name=f"pav{ch}_{dt}")
                    for dt in range(CT)
                ]
                # two independent accumulator chains (DVE + GpSimd) so the
                # 16 sden adds don't serialize on one engine
                acc_v = sb.tile([P, FB], F32R, tag="accs", bufs=2,
                                name=f"accv{ch}")
                acc_g = sb.tile([P, FB], F32R, tag="accg", bufs=2,
                                name=f"accg{ch}")
                pts = {}

                def emit_av(mp):
                    pt = pts.pop(mp)
                    for dt in range(CT):
                        nc.tensor.matmul(
                            pavs[dt],
                            vs[mp][:, :, dt * P:(dt + 1) * P],
                            pt,
                            start=(mp == 0), stop=(mp == MP - 1),
                            perf_mode=DR,
                        )

                for mi in range(NT):
                    psc = psp.tile([P, FB], F32, tag="psc", bufs=4,
                                   name=f"psc{ch}_{mi}")
                    for dp in range(CP):
                        nc.tensor.matmul(
                            psc,
                            kts[(dp, mi // 4)][:, :, (mi % 4) * P:(mi % 4 + 1) * P],
                            qts[(dp, ch)],
                            start=(dp == 0), stop=(dp == CP - 1),
                            perf_mode=DR,
                        )
                    if mi % 2 == 0:
                        pts[mi // 2] = sb.tile([P, 2, FB], FP8, tag="pt",
                                               bufs=16, name=f"pt{ch}_{mi // 2}")
                    dest = pts[mi // 2][:, mi % 2, :]
                    nc.scalar.activation(dest, psc, EXP,
                                         bias=bias_t, scale=SCALE)
                    if mi == 0:
                        nc.vector.tensor_copy(acc_v, dest)
                    elif mi == 1:
                        nc.gpsimd.tensor_copy(acc_g, dest)
                    elif mi % 2 == 0:
                        nc.vector.tensor_add(acc_v, acc_v, dest)
                    else:
                        nc.gpsimd.tensor_add(acc_g, acc_g, dest)
                    # AV lags the exp pipeline by one pair (two m-tiles) so
                    # it never stalls on ACT latency
                    if mi >= 3 and mi % 2 == 1:
                        emit_av((mi - 3) // 2)
                    if ch > 0:
                        if mi == 5:
                            emit_sden(ch - 1, prev_acc)
                        elif mi in (7, 9, 11, 13):
                            emit_proj_et(ch - 1, saved[ch - 1], (mi - 7) // 2)
                emit_av(MP - 1)

                nc.vector.tensor_add(acc_v, acc_v, acc_g)
                prev_acc = acc_v

                avts = []
                for dp in range(CP):
                    avts.append(sb.tile([P, 2, FB], FP8, tag="avt", bufs=4,
                                        name=f"avt{ch}_{dp}"))
                for dt in range(CT):
                    # pair-parallel: ACT does the even half, DVE the odd
                    dest = avts[dt // 2][:, dt % 2, :]
                    if dt % 2 == 0:
                        nc.scalar.mul(dest, pavs[dt], AV_SCALE)
                    else:
                        nc.vector.tensor_scalar_mul(dest, pavs[dt], AV_SCALE)
                saved[ch] = avts

            # tail: last chunk's proj, grouped dp-outer so the first four
            # matmuls only wait on the first avdr pair
            last = NCH - 1
            emit_sden(last, prev_acc)
            pys = [psp.tile([P, FB], F32, tag="psc" if et < 3 else "pav",
                            bufs=4, name=f"py{et}_{last}") for et in range(CT)]
            for dp in range(CP):
                for et in range(CT):
                    nc.tensor.matmul(
                        pys[et],
                        wp[dp][:, :, et * P:(et + 1) * P],
                        saved[last][dp],
                        start=(dp == 0), stop=(dp == CP - 1),
                        perf_mode=DR,
                    )
            for et in range(CT):
                yt = sb.tile([P, FB], BF16, tag="yo", bufs=3,
                             name=f"yt{et}_{last}")
                if et % 2 == 1:
                    nc.scalar.copy(yt, pys[et])
                else:
                    nc.vector.tensor_copy(yt, pys[et])
                nc.sync.dma_start(
                    yT[et * P:(et + 1) * P, last * FB:(last + 1) * FB], yt)

    nc.compile()
    return nc


def _pack_pairs(a):
    """[512, F] -> [256, 2F] c-pair layout (rows cp*256+j*128+p)."""
    f = a.shape[1]
    return np.ascontiguousarray(
        a.reshape(2, 2, P, f).transpose(0, 2, 1, 3).reshape(2 * P, 2 * f))


def _pack_x(xT):
    """[512, 2048] -> [256, 4096]: per chunk ch, cols ch*1024+j*512+f."""
    return np.ascontiguousarray(
        xT.reshape(2, 2, P, NCH, FB).transpose(0, 2, 3, 1, 4)
        .reshape(2 * P, NCH * 2 * FB))


def _prep_in_maps(x, w_qkv):
    wq = _pack_pairs(w_qkv[0:C].T.astype(np.float32)).astype(NP_FP8)
    wk = _pack_pairs(w_qkv[C:2 * C].T.astype(np.float32)).astype(NP_FP8)
    wv = _pack_pairs(w_qkv[2 * C:3 * C].T.astype(np.float32)).astype(NP_FP8)
    in_maps = []
    for b in range(B):
        in_maps.append({
            "xdr": _pack_x(x[b].T.astype(np.float32)).astype(NP_FP8),
            "wqd": wq, "wkd": wk, "wvd": wv,
        })
    return in_maps


_NC = None


def _get_nc():
    global _NC
    if _NC is None:
        _NC = build()
    return _NC


def kernel(x, w_qkv, w_proj, b_proj):
    x = np.asarray(x, dtype=np.float32)
    w_qkv = np.asarray(w_qkv, dtype=np.float32)
    w_proj = np.asarray(w_proj, dtype=np.float32)
    b_proj = np.asarray(b_proj, dtype=np.float32)

    in_maps = _prep_in_maps(x, w_qkv)
    wpp = _pack_pairs(w_proj.T).astype(NP_FP8)
    for m in in_maps:
        m["wpd"] = wpp

    nc = _get_nc()
    res = None
    for attempt in range(3):
        try:
            res = run_bass_kernel_spmd(nc, in_maps, core_ids=list(range(B)))
            break
        except Exception:
            if attempt == 2:
                raise
            import time
            time.sleep(5)

    wv_f32 = w_qkv[2 * C:3 * C]
    out = np.empty((B, N, C), np.float32)
    for b in range(B):
        r = res.results[b]
        s = r["sden"].reshape(N, 1)
        yt = np.asarray(r["yT"]).astype(np.float32)
        out[b] = yt.T * (1.0 / AV_SCALE) / s + (x[b] @ wv_f32.T) + b_proj[None, :]
    return out
